# revision 35
# baseline (speedup 1.0000x reference)
"""Multi-head attention (2-axis RoPE) Trainium2 kernel, 8-core data parallel.

Problem (hardcoded): B=16, S=1024 (32x32 grid), E=256, H=8, D=32, fp32 I/O.
  qkv = x @ Wqkv + bqkv ; RoPE(q), RoPE(k) ; softmax(q k^T / sqrt(D)) @ v ; @ Wout + bout

Sharding: batch across 8 cores (2 batches/core). Host scatters inputs /
gathers outputs; each core runs the full attention for its 2 batches.

v4 design notes (per core, T=2048 tokens):
  - single unified PSUM pool for the whole kernel: one 2-bank "big" ring
    (bufs=3) shared by qkv-proj, rope-permute, score and out-proj tiles +
    o_ps/den_ps accumulators (8 banks total).  No phase-boundary pool
    swap, so projection, rope and attention emission interleave freely.
  - emission schedule starts attention for batch 0 as soon as its q/k
    chunks are roped and its v tiles are projected (~1/3 into phase 1);
    the remaining projection units co-emit inside early attention
    iterations, hiding them under exp work.
  - exp split ScalarE (table exp) / VectorE (custom EXP16 op); rope
    cos-mul runs on the otherwise idle GpSimd engine; v-proj bias is a
    DVE add (kills the K=1 ones-row matmuls).
  - all matmul operands bf16; warmup matmuls keep the PE HAM
    un-throttled through the initial DMA window.
  - scores transposed [sk, sq], 4 heads packed as 32-row-band concurrent
    matmuls; AV/den as 4-head col-band concurrent matmuls; one
    reciprocal_approx_fast + tensor_mul normalizes 4 heads at once.
  - out-proj tiles queue right after their (b, half) is normalized, so
    they spread through attention instead of draining in a tail.
"""

import math

import numpy as np

B, G, H, D, E = 16, 32, 8, 32, 256
S = G * G
NCORES = 8
B_LOC = B // NCORES
T = B_LOC * S  # tokens per core
SCALE = 1.0 / math.sqrt(D)

# EXP16 constants: exp(x*SCALE) ~= ((x*C0p + C1p)^2 + C2p)^16
EXP_N = 16
C0P = SCALE / (EXP_N * math.sqrt(2.0))
C1P = 1.0 / math.sqrt(2.0)
C2P = 0.5

# which of the 16 half-tiles (t = 2*j + p) per (b, g, half) iteration are
# exp'd on the DVE (rest on ScalarE); alternate so both engines stay fed.
# Two sets used on alternating iterations give a 6.5/16 average DVE share
# (DVE also carries rope adds, v-bias adds and the softmax normalize).
DVE_TILES_A = frozenset((1, 3, 5, 7, 9, 11, 13))
DVE_TILES_B = frozenset((1, 3, 5, 7, 9, 11, 13, 15))

N_WARMUP = 120  # dummy PE matmuls issued at t=0 (HAM warmup during DMA)

_COMPILED = None
_LAST_RESULT = None  # test.py reads exec_time_ns / trace path from here


def _bf16(a):
    import ml_dtypes

    return np.asarray(a).astype(ml_dtypes.bfloat16)


def _rope_tables():
    """cos/sin [128, S] feature-major (row p multiplies feature d = p % 32 of
    every head; identical for every batch).  Device computes
    rope(x) = x*cos + P(x*sin) with P the pair swap (p ^ 1); the sign
    pattern sits pre-permutation: even rows +sin, odd rows -sin."""
    freqs = 1.0 / (10000.0 ** (np.arange(0, D, 4, dtype=np.float64) / D))  # [8]
    t = np.arange(G, dtype=np.float64)
    fx = t[:, None] * freqs[None, :]  # [32, 8]
    ax = np.broadcast_to(fx[:, None, :], (G, G, D // 4))
    ay = np.broadcast_to(fx[None, :, :], (G, G, D // 4))
    ang = np.concatenate([ax, ay], axis=-1).reshape(S, D // 2)  # [1024, 16]
    cos = np.cos(ang).astype(np.float32)  # [S, 16]
    sin = np.sin(ang).astype(np.float32)
    p = np.arange(128)
    pair = (p % D) // 2  # [128]
    sgn = np.where(p % 2 == 0, 1.0, -1.0).astype(np.float32)
    cosT = np.ascontiguousarray(cos[:, pair].T)  # [128, S]
    sinT = np.ascontiguousarray(sin[:, pair].T * sgn[:, None])
    return cosT, sinT


def _exp16_ref(in0, in1, c0, c1, c2):
    x = in0.astype(np.float32)
    u = (x * np.float32(c0) + np.float32(c1)).astype(np.float32)
    u = (u * u + np.float32(c2)).astype(np.float32)
    for _ in range(4):
        u = (u * u).astype(np.float32)
    return u


def _register_exp16():
    """Register the EXP16 custom DVE op (one 8-stage instruction) in
    concourse.dve_ops so the per-NEFF table generator and CoreSim see it."""
    import concourse.dve_ops as dops
    from concourse.dve_spec import C0, C1, C2, Spec, Src0, lower, sq
    from concourse.dve_uop import DveOpSpec

    name = "EXP16_MHA"
    for o in dops.OPS:
        if o.name == name:
            return o
    u = sq(Src0 * C0 + C1) + C2
    for _ in range(4):
        u = sq(u)
    spec = Spec(body=u, reference=_exp16_ref)
    row = max(dops._SUB_OPCODE_FOR_NAME.values()) + 1
    assert row < 0x20
    shas = {}
    for ver in ("v3", "v4"):
        shas[ver] = DveOpSpec(
            name=name, opcode=row, uops=lower(spec, ver=ver), rd1_en=False
        ).sha(ver)
    op = dops.DveOp(name, spec, subdim=False, uops_sha=shas)
    dops.OPS.append(op)
    dops._SUB_OPCODE_FOR_NAME[name] = row
    dops.CUSTOM_DVE_SPECS[name] = spec
    return op


def _build():
    import concourse.bass as bass  # noqa: F401
    import concourse.tile as tile
    from concourse import bacc, mybir

    f32 = mybir.dt.float32
    bf16 = mybir.dt.bfloat16
    exp16_op = _register_exp16()

    nc = bacc.Bacc("TRN2", target_bir_lowering=False, debug=False, num_devices=NCORES)

    # inputs packed into a few blobs: one DMA each (15 separate DMAs cost
    # ~10us of serialized descriptor-gen on the Sync queue at startup)
    xblob_d = nc.dram_tensor("xblob", [128, 2 * T], bf16, kind="ExternalInput").ap()
    wblob_d = nc.dram_tensor("wblob", [128, 2048], bf16, kind="ExternalInput").ap()
    cblob_d = nc.dram_tensor("cblob", [128, 3208], bf16, kind="ExternalInput").ap()
    oblob_d = nc.dram_tensor("oblob", [1, T + E], bf16, kind="ExternalInput").ap()
    out_d = nc.dram_tensor("out", [T, E], f32, kind="ExternalOutput").ap()

    with tile.TileContext(nc) as tc:
        consts = tc.alloc_tile_pool(name="consts", bufs=1)
        work = tc.alloc_tile_pool(name="work", bufs=1)
        ps = tc.alloc_tile_pool(name="ps", bufs=1, space="PSUM")

        ones_blk = consts.tile([128, 32], bf16, name="ones_blk")
        nc.vector.memset(ones_blk, 1.0)

        xblob = consts.tile([128, 2 * T], bf16, name="xblob")
        wblob = consts.tile([128, 2048], bf16, name="wblob")
        cblob = consts.tile([128, 3208], bf16, name="cblob")
        oblob = consts.tile([1, T + E], bf16, name="oblob")

        # two issue queues in parallel (Sync + Scalar HWDGE), ordered so the
        # first qk units' operands land first, then the cast bias/rope
        # tables, then the rest of x + wv/wo for the v/out projections
        nc.scalar.dma_start(out=xblob[:, 0:1024], in_=xblob_d[:, 0:1024])
        nc.sync.dma_start(out=wblob[:, 0:1024], in_=wblob_d[:, 0:1024])
        nc.scalar.dma_start(out=xblob[:, T : T + 1024],
                            in_=xblob_d[:, T : T + 1024])
        nc.sync.dma_start(out=cblob[:, 0:1160], in_=cblob_d[:, 0:1160])
        nc.scalar.dma_start(out=xblob[:, 1024:T], in_=xblob_d[:, 1024:T])
        nc.sync.dma_start(out=cblob[:, 1160:3208], in_=cblob_d[:, 1160:3208])
        nc.sync.dma_start(out=wblob[:, 1024:2048], in_=wblob_d[:, 1024:2048])
        nc.sync.dma_start(out=xblob[:, T + 1024 : 2 * T],
                          in_=xblob_d[:, T + 1024 : 2 * T])
        nc.sync.dma_start(out=oblob, in_=oblob_d)

        # preload the ScalarE activation tables (Exp + Identity-with-bias)
        # during the DMA window so the first cast doesn't eat the 1.3us
        # ACT_TABLE_LOAD on the critical path
        tdum = work.tile([1, 2], f32, name="tdum", tag="tdum", bufs=2)
        nc.scalar.activation(out=tdum, in_=ones_blk[0:1, 0:2],
                             func=mybir.ActivationFunctionType.Exp, scale=SCALE)
        tdum2 = work.tile([1, 2], f32, name="tdum2", tag="tdum", bufs=2)
        nc.scalar.add(out=tdum2, in_=ones_blk[0:1, 0:2], add=1.0)

        xT_chunks = [xblob[:, 0:T], xblob[:, T : 2 * T]]
        wqk_chunks = [wblob[:, 0:512], wblob[:, 512:1024]]
        wv_chunks = [wblob[:, 1024:1280], wblob[:, 1280:1536]]
        wo_chunks = [wblob[:, 1536:1792], wblob[:, 1792:2048],
                     oblob[0:1, T : T + E]]
        xT_ones = oblob[0:1, 0:T]
        qkbT = cblob[:, 0:8].bitcast(f32)
        pswap = cblob[:, 8:136]
        sinT = cblob[:, 136:1160]
        cosT = cblob[:, 1160:2184]
        vbias2 = cblob[:, 2184:3208].bitcast(f32).rearrange(
            "p (a b) -> p a b", a=2
        )

        # feature-major roped q/k: 4 chunks of 128 rows (q heads 0-7, k 0-7)
        qk_rope = [
            consts.tile([128, T], bf16, name=f"qk_rope{m}", tag=f"qk_rope{m}")
            for m in range(4)
        ]
        # v token-major: [128 tok, tok_tile, head, 32] bf16
        v_all = consts.tile([128, T // 128, H, D], bf16, name="v_all")
        # attention output, feature-major bf16: 2 chunks of 128 rows
        att_oT = [
            consts.tile([128, T], bf16, name=f"att_oT{g}", tag=f"att_oT{g}")
            for g in range(2)
        ]

        def big(name):
            """One 2-bank PSUM ring slot ([128, 2, 512] f32, 3-deep ring)."""
            return ps.tile([128, 2, 512], f32, name=name, tag="big", bufs=3)

        # HAM warmup: keep the PE busy while input DMAs stream so real
        # matmuls start at 2.4 GHz instead of the cold 1.2 GHz.
        warm = big("warm")
        with nc.named_scope("warm"):
            for _ in range(N_WARMUP):
                nc.tensor.matmul(
                    out=warm[0:32, 0, 0:32], lhsT=ones_blk, rhs=ones_blk[:, 0:32],
                    start=True, stop=True,
                )

        # ================= projection / rope units ========================
        pend1 = []

        def qk_unit(m, n, gp=False):
            sl = slice(n * 1024, n * 1024 + 1024)
            qk_ps = big("qk_ps")
            with nc.named_scope("qkproj"):
                for hv in range(2):  # N=512 matmul/PSUM-bank limit
                    xsl = slice(n * 1024 + hv * 512, n * 1024 + hv * 512 + 512)
                    for k in range(2):
                        nc.tensor.matmul(
                            out=qk_ps[:, hv, :],
                            lhsT=wqk_chunks[k][:, m * 128 : (m + 1) * 128],
                            rhs=xT_chunks[k][:, xsl],
                            start=(k == 0),
                            stop=(k == 1),
                        )
            with nc.named_scope("rope"):
                qk_bf = work.tile([128, 1024], bf16, name="qk_bf",
                                  tag="qk_bf", bufs=3)
                # cast + per-feature qkv bias in one ScalarE pass
                nc.scalar.add(out=qk_bf, in_=qk_ps.rearrange("p a b -> p (a b)"),
                              add=qkbT[:, m : m + 1])
                # co-emitted units run their rope muls on the idle GpSimd
                # (slow but off the exp engines; their tails pop several
                # j-slots later, hiding the latency).  Prefix units stay on
                # the DVE: they gate the very first scores.
                mul_eng = nc.gpsimd if gp else nc.vector
                t_sin = work.tile([128, 1024], bf16, name="t_sin",
                                  tag="t_sin", bufs=2)
                mul_eng.tensor_mul(t_sin, qk_bf, sinT)
                t_cos = work.tile([128, 1024], bf16, name="t_cos",
                                  tag="t_cos", bufs=2)
                mul_eng.tensor_mul(t_cos, qk_bf, cosT)

            def tail(m=m, sl=sl, t_sin=t_sin, t_cos=t_cos):
                with nc.named_scope("rope"):
                    perm_ps = big("perm_ps")
                    for hv in range(2):
                        nc.tensor.matmul(
                            out=perm_ps[:, hv, :], lhsT=pswap,
                            rhs=t_sin[:, hv * 512 : hv * 512 + 512],
                            start=True, stop=True,
                        )
                    nc.vector.tensor_add(
                        qk_rope[m][:, sl], t_cos,
                        perm_ps.rearrange("p a b -> p (a b)"),
                    )

            pend1.append(tail)

        def pop1():
            if pend1:
                pend1.pop(0)()

        def v_pair(u):
            """Project v token tiles 2u, 2u+1 (two 1-bank accumulators in one
            ring slot), then bias-add + bf16-cast both in one DVE pass."""
            with nc.named_scope("vproj"):
                v_ps = big("v_ps")
                for i in range(2):
                    tt = 2 * u + i
                    tsl = slice(tt * 128, (tt + 1) * 128)
                    for k in range(2):
                        nc.tensor.matmul(
                            out=v_ps[:, i, 0:E],
                            lhsT=xT_chunks[k][:, tsl],
                            rhs=wv_chunks[k],
                            start=(k == 0),
                            stop=(k == 1),
                        )
                nc.vector.tensor_add(
                    out=v_all[:, 2 * u : 2 * u + 2, :, :].rearrange(
                        "p t h d -> p t (h d)"
                    ),
                    in0=v_ps[:, :, 0:E],
                    in1=vbias2,
                )

        # ================= attention (software-pipelined) =================
        pending = []  # deferred emission closures (av/den units, norms)

        def drain_to(nmax, max_pop=2):
            popped = 0
            while len(pending) > nmax and popped < max_pop:
                pending.pop(0)()
                popped += 1

        def make_iter(b, g, half, dve_tiles):
            qc = qk_rope[g]
            kc = qk_rope[2 + g]
            qsl = slice(b * S + half * 512, b * S + half * 512 + 512)
            o_ps = ps.tile([128, 512], f32, name="o_ps", tag="o_ps", bufs=1)
            den_ps = ps.tile(
                [128, 512], f32, name="den_ps", tag="den_ps", bufs=1
            )
            tiles = {}

            def score_exp(j):
                ksl = slice(b * S + j * 128, b * S + j * 128 + 128)
                for p in range(2):
                    s_ps = big("s_ps")
                    with nc.named_scope("score"):
                        for e in range(2):
                            hl = 2 * p + e
                            psl = slice(32 * hl, 32 * hl + 32)
                            nc.tensor.matmul(
                                out=s_ps[:, e, :],
                                lhsT=kc[psl, ksl],
                                rhs=qc[psl, qsl],
                                start=True,
                                stop=True,
                                tile_position=(32 * hl, 0),
                            )
                    at = work.tile(
                        [128, 2, 512], bf16, name="attn", tag="attn", bufs=10
                    )
                    if 2 * j + p in dve_tiles:
                        with nc.named_scope("exp_dve"):
                            nc.vector._custom_dve(
                                exp16_op, out=at, in0=s_ps,
                                s0=C0P, s1=C1P, imm2=C2P,
                            )
                    else:
                        with nc.named_scope("exp_sc"):
                            nc.scalar.activation(
                                out=at,
                                in_=s_ps,
                                func=mybir.ActivationFunctionType.Exp,
                                scale=SCALE,
                            )
                    tiles[(j, p)] = at

            def av_den(j):
                # 4 AV matmuls (distinct col groups -> concurrent), then
                # 4 den matmuls; interleaving would serialize col groups.
                with nc.named_scope("av"):
                    for p in range(2):
                        at = tiles[(j, p)]
                        for e in range(2):
                            hl = 2 * p + e
                            osl = slice(32 * hl, 32 * hl + 32)
                            nc.tensor.matmul(
                                out=o_ps[osl, :],
                                lhsT=v_all[:, b * 8 + j, 4 * g + hl, :],
                                rhs=at[:, e, :],
                                start=(j == 0),
                                stop=(j == 7),
                                tile_position=(0, 32 * hl),
                            )
                with nc.named_scope("den"):
                    for p in range(2):
                        at = tiles[(j, p)]
                        for e in range(2):
                            hl = 2 * p + e
                            osl = slice(32 * hl, 32 * hl + 32)
                            nc.tensor.matmul(
                                out=den_ps[osl, :],
                                lhsT=ones_blk,
                                rhs=at[:, e, :],
                                start=(j == 0),
                                stop=(j == 7),
                                tile_position=(0, 32 * hl),
                            )

            def norm():
                with nc.named_scope("norm"):
                    bc = work.tile(
                        [128, 512], f32, name="bc", tag="bc", bufs=2
                    )
                    nc.vector.reciprocal_approx_fast(out=bc, in_=den_ps)
                    nc.vector.tensor_mul(att_oT[g][:, qsl], o_ps, bc)

            return score_exp, av_den, norm

        def out_proj_tile(tt):
            """One out-projection token tile; borrows a ring slot so it can
            interleave with attention iterations."""
            with nc.named_scope("outproj"):
                tsl = slice(tt * 128, (tt + 1) * 128)
                f = big("f_ps")
                f_ps = f[:, 0, 0:E]
                for k in range(3):
                    lhsT = (att_oT[0], att_oT[1], xT_ones)[k][:, tsl]
                    nc.tensor.matmul(
                        out=f_ps,
                        lhsT=lhsT,
                        rhs=wo_chunks[k],
                        start=(k == 0),
                        stop=(k == 2),
                    )
                o_sb = work.tile(
                    [128, E], f32, name="o_sb", tag="o_sb", bufs=4
                )
                nc.scalar.copy(out=o_sb, in_=f_ps)  # DVE is the busier engine
                nc.sync.dma_start(out=out_d[tsl, :], in_=o_sb)

        # ================= emission schedule ==============================
        # minimal prefix: rope just the two g0 qk chunks of batch 0, then
        # start attention immediately; every other projection/rope unit
        # co-emits inside early attention iterations (keyed by j slot).
        qk_unit(0, 0)
        qk_unit(2, 0)
        pop1()  # rope tail (0,0)
        pop1()  # rope tail (2,0)

        CO = {
            (0, 0, 0): {0: lambda: v_pair(0), 1: lambda: qk_unit(1, 0, gp=True),
                        2: lambda: v_pair(1), 3: lambda: qk_unit(3, 0, gp=True),
                        4: lambda: v_pair(2), 5: pop1,  # tail (1,0)
                        6: lambda: v_pair(3), 7: pop1},  # tail (3,0)
            (0, 0, 1): {1: lambda: qk_unit(0, 1, gp=True), 3: lambda: v_pair(4),
                        5: lambda: qk_unit(2, 1, gp=True)},
            (0, 1, 0): {1: pop1,  # tail (0,1)
                        3: lambda: qk_unit(1, 1, gp=True), 5: lambda: v_pair(5)},
            (0, 1, 1): {1: pop1,  # tail (2,1)
                        3: lambda: qk_unit(3, 1, gp=True), 5: lambda: v_pair(6)},
            (1, 0, 0): {1: pop1,  # tail (1,1)
                        3: lambda: v_pair(7), 5: pop1},  # tail (3,1)
        }
        for b in range(B_LOC):
            for half in range(2):
                for g in range(2):
                    co = CO.get((b, half, g), {})
                    it = b * 4 + half * 2 + g
                    dve_tiles = DVE_TILES_A if it % 2 == 0 else DVE_TILES_B
                    score_exp, av_den, norm = make_iter(b, g, half, dve_tiles)
                    for j in range(8):
                        score_exp(j)
                        pending.append(
                            (lambda f=av_den, jj=j: f(jj))
                        )
                        drain_to(2)
                        if j in co:
                            co[j]()
                    pending.append(norm)
                    # token tile tt needs both g-groups of its (b, half);
                    # queue its out-proj as soon as the second g completes
                    # so the tiles spread instead of draining in a tail
                    if g == 1:
                        t0 = b * 8 + half * 4
                        for tt in range(t0, t0 + 4):
                            pending.append(lambda t=tt: out_proj_tile(t))
        while pending:
            pending.pop(0)()

        work.release()
        consts.release()
        ps.release()

    nc.compile()
    return nc


def _fp8(a):
    import ml_dtypes

    return np.asarray(a).astype(ml_dtypes.float8_e4m3fn)


def _prep_core_inputs(x_loc, Wqkv, bqkv, Wout, bout, cosT, sinT, pswap):
    import ml_dtypes

    xT = x_loc.reshape(T, E).T.astype(np.float32)  # [256, T]
    wqk = Wqkv[:, : 2 * E]
    wv = Wqkv[:, 2 * E :]
    qkbT = np.ascontiguousarray(
        bqkv[: 2 * E].reshape(4, 128).T.astype(np.float32)
    )  # column m = bias for qk feature chunk m
    vbias2 = np.broadcast_to(bqkv[None, None, 2 * E :], (128, 2, E)).astype(
        np.float32
    )
    xblob = np.concatenate([_bf16(xT[0:128]), _bf16(xT[128:256])], axis=1)
    wblob = np.concatenate(
        [_bf16(wqk[0:128]), _bf16(wqk[128:256]),
         _bf16(wv[0:128]), _bf16(wv[128:256]),
         _bf16(Wout[0:128]), _bf16(Wout[128:256])],
        axis=1,
    )
    u16 = lambda a: np.ascontiguousarray(a).view(np.uint16).reshape(128, -1)
    cblob = np.concatenate(
        [u16(qkbT), u16(_bf16(pswap)), u16(_bf16(sinT)), u16(_bf16(cosT)),
         u16(vbias2)],
        axis=1,
    ).view(ml_dtypes.bfloat16)
    oblob = np.concatenate(
        [np.ones((1, T), np.float32), bout[None, :].astype(np.float32)], axis=1
    )
    return {
        "xblob": np.ascontiguousarray(xblob),
        "wblob": np.ascontiguousarray(wblob),
        "cblob": np.ascontiguousarray(cblob),
        "oblob": np.ascontiguousarray(_bf16(oblob)),
    }


def _pswap_mat():
    p = np.zeros((128, 128), np.float32)
    idx = np.arange(128)
    p[idx, idx ^ 1] = 1.0
    return p


def kernel(x, Wqkv, bqkv, Wout, bout):
    global _COMPILED, _LAST_RESULT
    from concourse.bass_utils import run_bass_kernel_spmd

    if _COMPILED is None:
        _COMPILED = _build()
    nc = _COMPILED

    x = np.asarray(x, np.float32)
    Wqkv = np.asarray(Wqkv, np.float32)
    bqkv = np.asarray(bqkv, np.float32)
    Wout = np.asarray(Wout, np.float32)
    bout = np.asarray(bout, np.float32)

    cosT, sinT = _rope_tables()
    pswap = _pswap_mat()

    in_maps = [
        _prep_core_inputs(
            x[c * B_LOC : (c + 1) * B_LOC], Wqkv, bqkv, Wout, bout, cosT, sinT, pswap
        )
        for c in range(NCORES)
    ]
    res = run_bass_kernel_spmd(nc, in_maps, list(range(NCORES)))
    _LAST_RESULT = res
    out = np.stack([res.results[c]["out"].reshape(B_LOC, S, E) for c in range(NCORES)])
    return np.ascontiguousarray(out.reshape(B, S, E))


# ---------------------------------------------------------------------------
# host model: numpy mirror of the device dataflow (bf16 casts, EXP16 tiles)
def host_model(x, Wqkv, bqkv, Wout, bout):
    def f32(a):
        return np.asarray(a, np.float32)

    cosT, sinT = _rope_tables()
    cosT_b = f32(_bf16(cosT))
    sinT_b = f32(_bf16(sinT))
    perm = np.arange(128) ^ 1
    outs = []
    for c in range(NCORES):
        x_loc = f32(x)[c * B_LOC : (c + 1) * B_LOC]
        xT = f32(_bf16(x_loc.reshape(T, E).T))
        wqk = f32(_bf16(Wqkv[:, : 2 * E]))
        wv = f32(_bf16(Wqkv[:, 2 * E :]))
        wo = f32(_bf16(Wout))
        wob = f32(_bf16(np.asarray(bout, np.float32)))
        qkb = f32(bqkv)[: 2 * E].reshape(4, 128).T
        vb = f32(bqkv)[2 * E :]
        qkT = wqk.T @ xT  # [512, T] fp32 accum of bf16 operands
        qkr = np.empty((512, T), np.float32)
        cs2 = np.tile(cosT_b, (1, B_LOC))
        sn2 = np.tile(sinT_b, (1, B_LOC))
        for mm in range(4):
            # ScalarE cast + per-feature bias
            ch = f32(_bf16(qkT[mm * 128 : (mm + 1) * 128] + qkb[:, mm : mm + 1]))
            t_sin = f32(_bf16(ch * sn2))
            t_cos = f32(_bf16(ch * cs2))
            qkr[mm * 128 : (mm + 1) * 128] = f32(_bf16(t_cos + t_sin[perm, :]))
        # v proj (fp32 psum of bf16 matmul) + f32 bias, cast bf16
        v = f32(_bf16(xT.T @ wv + vb))
        v = v.reshape(T, H, D)  # token-major bf16
        att_oT = np.empty((256, T), np.float32)
        for b in range(B_LOC):
            for g in range(2):
                for half in range(2):
                    it = b * 4 + half * 2 + g
                    dve_tiles = DVE_TILES_A if it % 2 == 0 else DVE_TILES_B
                    qsl = slice(b * S + half * 512, b * S + half * 512 + 512)
                    o_acc = np.zeros((128, 512), np.float32)
                    den_acc = np.zeros((4, 512), np.float32)
                    for j in range(8):
                        ksl = slice(b * S + j * 128, b * S + j * 128 + 128)
                        for p in range(2):
                            for e in range(2):
                                hl = 2 * p + e
                                psl = slice(g * 128 + 32 * hl, g * 128 + 32 * hl + 32)
                                kc = qkr[256 + psl.start : 256 + psl.stop, ksl]
                                qc = qkr[psl, qsl]
                                scores = kc.T @ qc  # [128, 512]
                                if 2 * j + p in dve_tiles:
                                    ex = _exp16_ref(scores, None, C0P, C1P, C2P)
                                else:
                                    ex = np.exp(scores * SCALE)
                                ex = f32(_bf16(ex))
                                vb_ = v[b * S + j * 128 : b * S + (j + 1) * 128,
                                        4 * g + hl]  # [128, 32]
                                o_acc[32 * hl : 32 * hl + 32] += vb_.T @ ex
                                den_acc[hl] += ex.sum(0)
                    bc = 1.0 / den_acc  # recip_approx ~ exact here
                    o_n = np.empty_like(o_acc)
                    for hl in range(4):
                        o_n[32 * hl : 32 * hl + 32] = (
                            o_acc[32 * hl : 32 * hl + 32] * bc[hl]
                        )
                    att_oT[g * 128 : (g + 1) * 128, qsl] = f32(_bf16(o_n))
        out = f32(_bf16(att_oT)).T @ wo + wob[None, :]
        outs.append(out.reshape(B_LOC, S, E))
    return np.concatenate(outs, 0).astype(np.float32)


# revision 39
# speedup vs baseline: 1.0250x; 1.0250x over previous
"""Multi-head attention (2-axis RoPE) Trainium2 kernel, 8-core data parallel.

Problem (hardcoded): B=16, S=1024 (32x32 grid), E=256, H=8, D=32, fp32 I/O.
  qkv = x @ Wqkv + bqkv ; RoPE(q), RoPE(k) ; softmax(q k^T / sqrt(D)) @ v ; @ Wout + bout

Sharding: batch across 8 cores (2 batches/core). Host scatters inputs /
gathers outputs; each core runs the full attention for its 2 batches.

v4 design notes (per core, T=2048 tokens):
  - single unified PSUM pool for the whole kernel: one 2-bank "big" ring
    (bufs=3) shared by qkv-proj, rope-permute, score and out-proj tiles +
    o_ps/den_ps accumulators (8 banks total).  No phase-boundary pool
    swap, so projection, rope and attention emission interleave freely.
  - emission schedule starts attention for batch 0 as soon as its q/k
    chunks are roped and its v tiles are projected (~1/3 into phase 1);
    the remaining projection units co-emit inside early attention
    iterations, hiding them under exp work.
  - exp split ScalarE (table exp) / VectorE (custom EXP16 op); rope
    cos-mul runs on the otherwise idle GpSimd engine; v-proj bias is a
    DVE add (kills the K=1 ones-row matmuls).
  - all matmul operands bf16; warmup matmuls keep the PE HAM
    un-throttled through the initial DMA window.
  - scores transposed [sk, sq], 4 heads packed as 32-row-band concurrent
    matmuls; AV/den as 4-head col-band concurrent matmuls; one
    reciprocal_approx_fast + tensor_mul normalizes 4 heads at once.
  - out-proj tiles queue right after their (b, half) is normalized, so
    they spread through attention instead of draining in a tail.
"""

import math

import numpy as np

B, G, H, D, E = 16, 32, 8, 32, 256
S = G * G
NCORES = 8
B_LOC = B // NCORES
T = B_LOC * S  # tokens per core
SCALE = 1.0 / math.sqrt(D)

# EXP16 constants: exp(x*SCALE) ~= ((x*C0p + C1p)^2 + C2p)^16
EXP_N = 16
C0P = SCALE / (EXP_N * math.sqrt(2.0))
C1P = 1.0 / math.sqrt(2.0)
C2P = 0.5

# which of the 16 half-tiles (t = 2*j + p) per (b, g, half) iteration are
# exp'd on the DVE (rest on ScalarE); alternate so both engines stay fed.
# Two sets used on alternating iterations give a 6.5/16 average DVE share
# (DVE also carries rope adds, v-bias adds and the softmax normalize).
# A puts the unavoidable Scalar/Scalar pair at j=0, overlapping the DVE's
# norm+recip of the previous iteration
DVE_TILES_A = frozenset((3, 5, 7, 9, 11, 13, 15))
DVE_TILES_B = frozenset((1, 3, 5, 7, 9, 11, 13, 15))

N_WARMUP = 120  # dummy PE matmuls issued at t=0 (HAM warmup during DMA)

_COMPILED = None
_LAST_RESULT = None  # test.py reads exec_time_ns / trace path from here


def _bf16(a):
    import ml_dtypes

    return np.asarray(a).astype(ml_dtypes.bfloat16)


def _rope_tables():
    """cos/sin [128, S] feature-major (row p multiplies feature d = p % 32 of
    every head; identical for every batch).  Device computes
    rope(x) = x*cos + P(x*sin) with P the pair swap (p ^ 1); the sign
    pattern sits pre-permutation: even rows +sin, odd rows -sin."""
    freqs = 1.0 / (10000.0 ** (np.arange(0, D, 4, dtype=np.float64) / D))  # [8]
    t = np.arange(G, dtype=np.float64)
    fx = t[:, None] * freqs[None, :]  # [32, 8]
    ax = np.broadcast_to(fx[:, None, :], (G, G, D // 4))
    ay = np.broadcast_to(fx[None, :, :], (G, G, D // 4))
    ang = np.concatenate([ax, ay], axis=-1).reshape(S, D // 2)  # [1024, 16]
    cos = np.cos(ang).astype(np.float32)  # [S, 16]
    sin = np.sin(ang).astype(np.float32)
    p = np.arange(128)
    pair = (p % D) // 2  # [128]
    sgn = np.where(p % 2 == 0, 1.0, -1.0).astype(np.float32)
    cosT = np.ascontiguousarray(cos[:, pair].T)  # [128, S]
    sinT = np.ascontiguousarray(sin[:, pair].T * sgn[:, None])
    return cosT, sinT


def _exp16_ref(in0, in1, c0, c1, c2):
    x = in0.astype(np.float32)
    u = (x * np.float32(c0) + np.float32(c1)).astype(np.float32)
    u = (u * u + np.float32(c2)).astype(np.float32)
    for _ in range(4):
        u = (u * u).astype(np.float32)
    return u


def _register_exp16():
    """Register the EXP16 custom DVE op (one 8-stage instruction) in
    concourse.dve_ops so the per-NEFF table generator and CoreSim see it."""
    import concourse.dve_ops as dops
    from concourse.dve_spec import C0, C1, C2, Spec, Src0, lower, sq
    from concourse.dve_uop import DveOpSpec

    name = "EXP16_MHA"
    for o in dops.OPS:
        if o.name == name:
            return o
    u = sq(Src0 * C0 + C1) + C2
    for _ in range(4):
        u = sq(u)
    spec = Spec(body=u, reference=_exp16_ref)
    row = max(dops._SUB_OPCODE_FOR_NAME.values()) + 1
    assert row < 0x20
    shas = {}
    for ver in ("v3", "v4"):
        shas[ver] = DveOpSpec(
            name=name, opcode=row, uops=lower(spec, ver=ver), rd1_en=False
        ).sha(ver)
    op = dops.DveOp(name, spec, subdim=False, uops_sha=shas)
    dops.OPS.append(op)
    dops._SUB_OPCODE_FOR_NAME[name] = row
    dops.CUSTOM_DVE_SPECS[name] = spec
    return op


def _build():
    import concourse.bass as bass  # noqa: F401
    import concourse.tile as tile
    from concourse import bacc, mybir

    f32 = mybir.dt.float32
    bf16 = mybir.dt.bfloat16
    exp16_op = _register_exp16()

    nc = bacc.Bacc("TRN2", target_bir_lowering=False, debug=False, num_devices=NCORES)

    # inputs packed into a few blobs: one DMA each (15 separate DMAs cost
    # ~10us of serialized descriptor-gen on the Sync queue at startup)
    xblob_d = nc.dram_tensor("xblob", [128, 2 * T], bf16, kind="ExternalInput").ap()
    wblob_d = nc.dram_tensor("wblob", [128, 2048], bf16, kind="ExternalInput").ap()
    cblob_d = nc.dram_tensor("cblob", [128, 3208], bf16, kind="ExternalInput").ap()
    oblob_d = nc.dram_tensor("oblob", [1, T + E], bf16, kind="ExternalInput").ap()
    out_d = nc.dram_tensor("out", [T, E], f32, kind="ExternalOutput").ap()

    with tile.TileContext(nc) as tc:
        consts = tc.alloc_tile_pool(name="consts", bufs=1)
        work = tc.alloc_tile_pool(name="work", bufs=1)
        ps = tc.alloc_tile_pool(name="ps", bufs=1, space="PSUM")

        ones_blk = consts.tile([128, 32], bf16, name="ones_blk")
        nc.vector.memset(ones_blk, 1.0)

        xblob = consts.tile([128, 2 * T], bf16, name="xblob")
        wblob = consts.tile([128, 2048], bf16, name="wblob")
        cblob = consts.tile([128, 3208], bf16, name="cblob")
        oblob = consts.tile([1, T + E], bf16, name="oblob")

        # two issue queues in parallel (Sync + Scalar HWDGE), ordered so the
        # first qk units' operands land first, then the cast bias/rope
        # tables, then the rest of x + wv/wo for the v/out projections
        nc.scalar.dma_start(out=xblob[:, 0:1024], in_=xblob_d[:, 0:1024])
        nc.sync.dma_start(out=wblob[:, 0:1024], in_=wblob_d[:, 0:1024])
        nc.scalar.dma_start(out=xblob[:, T : T + 1024],
                            in_=xblob_d[:, T : T + 1024])
        nc.sync.dma_start(out=cblob[:, 0:1160], in_=cblob_d[:, 0:1160])
        nc.scalar.dma_start(out=xblob[:, 1024:T], in_=xblob_d[:, 1024:T])
        nc.sync.dma_start(out=cblob[:, 1160:3208], in_=cblob_d[:, 1160:3208])
        nc.sync.dma_start(out=wblob[:, 1024:2048], in_=wblob_d[:, 1024:2048])
        nc.sync.dma_start(out=xblob[:, T + 1024 : 2 * T],
                          in_=xblob_d[:, T + 1024 : 2 * T])
        nc.sync.dma_start(out=oblob, in_=oblob_d)

        # preload the ScalarE activation tables (Exp + Identity-with-bias)
        # during the DMA window so the first cast doesn't eat the 1.3us
        # ACT_TABLE_LOAD on the critical path
        tdum = work.tile([1, 2], f32, name="tdum", tag="tdum", bufs=2)
        nc.scalar.activation(out=tdum, in_=ones_blk[0:1, 0:2],
                             func=mybir.ActivationFunctionType.Exp, scale=SCALE)
        tdum2 = work.tile([1, 2], f32, name="tdum2", tag="tdum", bufs=2)
        nc.scalar.add(out=tdum2, in_=ones_blk[0:1, 0:2], add=1.0)

        xT_chunks = [xblob[:, 0:T], xblob[:, T : 2 * T]]
        wqk_chunks = [wblob[:, 0:512], wblob[:, 512:1024]]
        wv_chunks = [wblob[:, 1024:1280], wblob[:, 1280:1536]]
        wo_chunks = [wblob[:, 1536:1792], wblob[:, 1792:2048],
                     oblob[0:1, T : T + E]]
        xT_ones = oblob[0:1, 0:T]
        qkbT = cblob[:, 0:8].bitcast(f32)
        pswap = cblob[:, 8:136]
        sinT = cblob[:, 136:1160]
        cosT = cblob[:, 1160:2184]
        vbias2 = cblob[:, 2184:3208].bitcast(f32).rearrange(
            "p (a b) -> p a b", a=2
        )

        # feature-major roped q/k: 4 chunks of 128 rows (q heads 0-7, k 0-7)
        qk_rope = [
            consts.tile([128, T], bf16, name=f"qk_rope{m}", tag=f"qk_rope{m}")
            for m in range(4)
        ]
        # v token-major: [128 tok, tok_tile, head, 32] bf16
        v_all = consts.tile([128, T // 128, H, D], bf16, name="v_all")
        # attention output, feature-major bf16: 2 chunks of 128 rows
        att_oT = [
            consts.tile([128, T], bf16, name=f"att_oT{g}", tag=f"att_oT{g}")
            for g in range(2)
        ]

        def big(name):
            """One 2-bank PSUM ring slot ([128, 2, 512] f32, 3-deep ring)."""
            return ps.tile([128, 2, 512], f32, name=name, tag="big", bufs=3)

        # HAM warmup: keep the PE busy while input DMAs stream so real
        # matmuls start at 2.4 GHz instead of the cold 1.2 GHz.
        warm = big("warm")
        with nc.named_scope("warm"):
            for _ in range(N_WARMUP):
                nc.tensor.matmul(
                    out=warm[0:32, 0, 0:32], lhsT=ones_blk, rhs=ones_blk[:, 0:32],
                    start=True, stop=True,
                )

        # ================= projection / rope units ========================
        pend1 = []

        def qk_unit(m, n, gp=False):
            sl = slice(n * 1024, n * 1024 + 1024)
            qk_ps = big("qk_ps")
            with nc.named_scope("qkproj"):
                for hv in range(2):  # N=512 matmul/PSUM-bank limit
                    xsl = slice(n * 1024 + hv * 512, n * 1024 + hv * 512 + 512)
                    for k in range(2):
                        nc.tensor.matmul(
                            out=qk_ps[:, hv, :],
                            lhsT=wqk_chunks[k][:, m * 128 : (m + 1) * 128],
                            rhs=xT_chunks[k][:, xsl],
                            start=(k == 0),
                            stop=(k == 1),
                        )
            with nc.named_scope("rope"):
                qk_bf = work.tile([128, 1024], bf16, name="qk_bf",
                                  tag="qk_bf", bufs=3)
                # cast + per-feature qkv bias in one ScalarE pass
                nc.scalar.add(out=qk_bf, in_=qk_ps.rearrange("p a b -> p (a b)"),
                              add=qkbT[:, m : m + 1])
                # co-emitted units run their rope muls on the idle GpSimd
                # (slow but off the exp engines; their tails pop several
                # j-slots later, hiding the latency).  Prefix units stay on
                # the DVE: they gate the very first scores.
                mul_eng = nc.gpsimd if gp else nc.vector
                t_sin = work.tile([128, 1024], bf16, name="t_sin",
                                  tag="t_sin", bufs=2)
                mul_eng.tensor_mul(t_sin, qk_bf, sinT)
                t_cos = work.tile([128, 1024], bf16, name="t_cos",
                                  tag="t_cos", bufs=2)
                mul_eng.tensor_mul(t_cos, qk_bf, cosT)

            def tail(m=m, sl=sl, t_sin=t_sin, t_cos=t_cos):
                with nc.named_scope("rope"):
                    perm_ps = big("perm_ps")
                    for hv in range(2):
                        nc.tensor.matmul(
                            out=perm_ps[:, hv, :], lhsT=pswap,
                            rhs=t_sin[:, hv * 512 : hv * 512 + 512],
                            start=True, stop=True,
                        )
                    nc.vector.tensor_add(
                        qk_rope[m][:, sl], t_cos,
                        perm_ps.rearrange("p a b -> p (a b)"),
                    )

            pend1.append(tail)

        def pop1():
            if pend1:
                pend1.pop(0)()

        def v_pair(u):
            """Project v token tiles 2u, 2u+1 (two 1-bank accumulators in one
            ring slot), then bias-add + bf16-cast both in one DVE pass."""
            with nc.named_scope("vproj"):
                v_ps = big("v_ps")
                for i in range(2):
                    tt = 2 * u + i
                    tsl = slice(tt * 128, (tt + 1) * 128)
                    for k in range(2):
                        nc.tensor.matmul(
                            out=v_ps[:, i, 0:E],
                            lhsT=xT_chunks[k][:, tsl],
                            rhs=wv_chunks[k],
                            start=(k == 0),
                            stop=(k == 1),
                        )
                nc.vector.tensor_add(
                    out=v_all[:, 2 * u : 2 * u + 2, :, :].rearrange(
                        "p t h d -> p t (h d)"
                    ),
                    in0=v_ps[:, :, 0:E],
                    in1=vbias2,
                )

        # ================= attention (software-pipelined) =================
        pending = []  # deferred emission closures (av/den units, norms)

        def drain_to(nmax, max_pop=2):
            popped = 0
            while len(pending) > nmax and popped < max_pop:
                pending.pop(0)()
                popped += 1

        def make_iter(b, g, half, dve_tiles):
            qc = qk_rope[g]
            kc = qk_rope[2 + g]
            qsl = slice(b * S + half * 512, b * S + half * 512 + 512)
            o_ps = ps.tile([128, 512], f32, name="o_ps", tag="o_ps", bufs=1)
            den_ps = ps.tile(
                [128, 512], f32, name="den_ps", tag="den_ps", bufs=1
            )
            tiles = {}

            def score_exp(j):
                ksl = slice(b * S + j * 128, b * S + j * 128 + 128)
                for p in range(2):
                    s_ps = big("s_ps")
                    with nc.named_scope("score"):
                        for e in range(2):
                            hl = 2 * p + e
                            psl = slice(32 * hl, 32 * hl + 32)
                            nc.tensor.matmul(
                                out=s_ps[:, e, :],
                                lhsT=kc[psl, ksl],
                                rhs=qc[psl, qsl],
                                start=True,
                                stop=True,
                                tile_position=(32 * hl, 0),
                            )
                    at = work.tile(
                        [128, 2, 512], bf16, name="attn", tag="attn", bufs=10
                    )
                    if 2 * j + p in dve_tiles:
                        with nc.named_scope("exp_dve"):
                            nc.vector._custom_dve(
                                exp16_op, out=at, in0=s_ps,
                                s0=C0P, s1=C1P, imm2=C2P,
                            )
                    else:
                        with nc.named_scope("exp_sc"):
                            nc.scalar.activation(
                                out=at,
                                in_=s_ps,
                                func=mybir.ActivationFunctionType.Exp,
                                scale=SCALE,
                            )
                    tiles[(j, p)] = at

            def av_den(j):
                # 4 AV matmuls (distinct col groups -> concurrent), then
                # 4 den matmuls; interleaving would serialize col groups.
                with nc.named_scope("av"):
                    for p in range(2):
                        at = tiles[(j, p)]
                        for e in range(2):
                            hl = 2 * p + e
                            osl = slice(32 * hl, 32 * hl + 32)
                            nc.tensor.matmul(
                                out=o_ps[osl, :],
                                lhsT=v_all[:, b * 8 + j, 4 * g + hl, :],
                                rhs=at[:, e, :],
                                start=(j == 0),
                                stop=(j == 7),
                                tile_position=(0, 32 * hl),
                            )
                with nc.named_scope("den"):
                    for p in range(2):
                        at = tiles[(j, p)]
                        for e in range(2):
                            hl = 2 * p + e
                            osl = slice(32 * hl, 32 * hl + 32)
                            nc.tensor.matmul(
                                out=den_ps[osl, :],
                                lhsT=ones_blk,
                                rhs=at[:, e, :],
                                start=(j == 0),
                                stop=(j == 7),
                                tile_position=(0, 32 * hl),
                            )

            def norm():
                with nc.named_scope("norm"):
                    bc = work.tile(
                        [128, 512], f32, name="bc", tag="bc", bufs=2
                    )
                    nc.vector.reciprocal_approx_fast(out=bc, in_=den_ps)
                    nc.vector.tensor_mul(att_oT[g][:, qsl], o_ps, bc)

            return score_exp, av_den, norm

        def out_proj_pair(tt):
            """Two out-projection token tiles in one ring slot (one bank
            each): one Scalar copy + one DMA for both."""
            with nc.named_scope("outproj"):
                f = big("f_ps")
                for i in range(2):
                    tsl = slice((tt + i) * 128, (tt + i + 1) * 128)
                    for k in range(3):
                        lhsT = (att_oT[0], att_oT[1], xT_ones)[k][:, tsl]
                        nc.tensor.matmul(
                            out=f[:, i, 0:E],
                            lhsT=lhsT,
                            rhs=wo_chunks[k],
                            start=(k == 0),
                            stop=(k == 2),
                        )
                o_sb = work.tile(
                    [128, 2, E], f32, name="o_sb", tag="o_sb", bufs=2
                )
                nc.scalar.copy(out=o_sb, in_=f[:, :, 0:E])
                nc.sync.dma_start(
                    out=out_d[tt * 128 : (tt + 2) * 128, :].rearrange(
                        "(i p) e -> p i e", p=128
                    ),
                    in_=o_sb,
                )

        # ================= emission schedule ==============================
        # minimal prefix: rope just the two g0 qk chunks of batch 0, then
        # start attention immediately; every other projection/rope unit
        # co-emits inside early attention iterations (keyed by j slot).
        qk_unit(0, 0)
        qk_unit(2, 0)
        pop1()  # rope tail (0,0)
        pop1()  # rope tail (2,0)

        CO = {
            (0, 0, 0): {0: lambda: v_pair(0), 1: lambda: qk_unit(1, 0, gp=True),
                        2: lambda: v_pair(1), 3: lambda: qk_unit(3, 0, gp=True),
                        4: lambda: v_pair(2), 5: pop1,  # tail (1,0)
                        6: lambda: v_pair(3), 7: pop1},  # tail (3,0)
            (0, 0, 1): {1: lambda: qk_unit(0, 1, gp=True), 3: lambda: v_pair(4),
                        5: lambda: qk_unit(2, 1, gp=True)},
            (0, 1, 0): {1: pop1,  # tail (0,1)
                        3: lambda: qk_unit(1, 1, gp=True), 5: lambda: v_pair(5)},
            (0, 1, 1): {1: pop1,  # tail (2,1)
                        3: lambda: qk_unit(3, 1, gp=True), 5: lambda: v_pair(6)},
            (1, 0, 0): {1: pop1,  # tail (1,1)
                        3: lambda: v_pair(7), 5: pop1},  # tail (3,1)
        }
        for b in range(B_LOC):
            for half in range(2):
                for g in range(2):
                    co = CO.get((b, half, g), {})
                    it = b * 4 + half * 2 + g
                    dve_tiles = DVE_TILES_A if it % 2 == 0 else DVE_TILES_B
                    score_exp, av_den, norm = make_iter(b, g, half, dve_tiles)
                    for j in range(8):
                        score_exp(j)
                        pending.append(
                            (lambda f=av_den, jj=j: f(jj))
                        )
                        drain_to(2)
                        if j in co:
                            co[j]()
                    pending.append(norm)
                    # token tile tt needs both g-groups of its (b, half);
                    # queue its out-proj as soon as the second g completes
                    # so the tiles spread instead of draining in a tail
                    if g == 1:
                        t0 = b * 8 + half * 4
                        pending.append(lambda t=t0: out_proj_pair(t))
                        pending.append(lambda t=t0 + 2: out_proj_pair(t))
        while pending:
            pending.pop(0)()

        work.release()
        consts.release()
        ps.release()

    nc.compile()
    return nc


def _prep_core_inputs(x_loc, Wqkv, bqkv, Wout, bout, cosT, sinT, pswap):
    import ml_dtypes

    xT = x_loc.reshape(T, E).T.astype(np.float32)  # [256, T]
    wqk = Wqkv[:, : 2 * E]
    wv = Wqkv[:, 2 * E :]
    qkbT = np.ascontiguousarray(
        bqkv[: 2 * E].reshape(4, 128).T.astype(np.float32)
    )  # column m = bias for qk feature chunk m
    vbias2 = np.broadcast_to(bqkv[None, None, 2 * E :], (128, 2, E)).astype(
        np.float32
    )
    xblob = np.concatenate([_bf16(xT[0:128]), _bf16(xT[128:256])], axis=1)
    wblob = np.concatenate(
        [_bf16(wqk[0:128]), _bf16(wqk[128:256]),
         _bf16(wv[0:128]), _bf16(wv[128:256]),
         _bf16(Wout[0:128]), _bf16(Wout[128:256])],
        axis=1,
    )
    u16 = lambda a: np.ascontiguousarray(a).view(np.uint16).reshape(128, -1)
    cblob = np.concatenate(
        [u16(qkbT), u16(_bf16(pswap)), u16(_bf16(sinT)), u16(_bf16(cosT)),
         u16(vbias2)],
        axis=1,
    ).view(ml_dtypes.bfloat16)
    oblob = np.concatenate(
        [np.ones((1, T), np.float32), bout[None, :].astype(np.float32)], axis=1
    )
    return {
        "xblob": np.ascontiguousarray(xblob),
        "wblob": np.ascontiguousarray(wblob),
        "cblob": np.ascontiguousarray(cblob),
        "oblob": np.ascontiguousarray(_bf16(oblob)),
    }


def _pswap_mat():
    p = np.zeros((128, 128), np.float32)
    idx = np.arange(128)
    p[idx, idx ^ 1] = 1.0
    return p


def kernel(x, Wqkv, bqkv, Wout, bout):
    global _COMPILED, _LAST_RESULT
    from concourse.bass_utils import run_bass_kernel_spmd

    if _COMPILED is None:
        _COMPILED = _build()
    nc = _COMPILED

    x = np.asarray(x, np.float32)
    Wqkv = np.asarray(Wqkv, np.float32)
    bqkv = np.asarray(bqkv, np.float32)
    Wout = np.asarray(Wout, np.float32)
    bout = np.asarray(bout, np.float32)

    cosT, sinT = _rope_tables()
    pswap = _pswap_mat()

    in_maps = [
        _prep_core_inputs(
            x[c * B_LOC : (c + 1) * B_LOC], Wqkv, bqkv, Wout, bout, cosT, sinT, pswap
        )
        for c in range(NCORES)
    ]
    res = run_bass_kernel_spmd(nc, in_maps, list(range(NCORES)))
    _LAST_RESULT = res
    out = np.stack([res.results[c]["out"].reshape(B_LOC, S, E) for c in range(NCORES)])
    return np.ascontiguousarray(out.reshape(B, S, E))


# ---------------------------------------------------------------------------
# host model: numpy mirror of the device dataflow (bf16 casts, EXP16 tiles)
def host_model(x, Wqkv, bqkv, Wout, bout):
    def f32(a):
        return np.asarray(a, np.float32)

    cosT, sinT = _rope_tables()
    cosT_b = f32(_bf16(cosT))
    sinT_b = f32(_bf16(sinT))
    perm = np.arange(128) ^ 1
    outs = []
    for c in range(NCORES):
        x_loc = f32(x)[c * B_LOC : (c + 1) * B_LOC]
        xT = f32(_bf16(x_loc.reshape(T, E).T))
        wqk = f32(_bf16(Wqkv[:, : 2 * E]))
        wv = f32(_bf16(Wqkv[:, 2 * E :]))
        wo = f32(_bf16(Wout))
        wob = f32(_bf16(np.asarray(bout, np.float32)))
        qkb = f32(bqkv)[: 2 * E].reshape(4, 128).T
        vb = f32(bqkv)[2 * E :]
        qkT = wqk.T @ xT  # [512, T] fp32 accum of bf16 operands
        qkr = np.empty((512, T), np.float32)
        cs2 = np.tile(cosT_b, (1, B_LOC))
        sn2 = np.tile(sinT_b, (1, B_LOC))
        for mm in range(4):
            # ScalarE cast + per-feature bias
            ch = f32(_bf16(qkT[mm * 128 : (mm + 1) * 128] + qkb[:, mm : mm + 1]))
            t_sin = f32(_bf16(ch * sn2))
            t_cos = f32(_bf16(ch * cs2))
            qkr[mm * 128 : (mm + 1) * 128] = f32(_bf16(t_cos + t_sin[perm, :]))
        # v proj (fp32 psum of bf16 matmul) + f32 bias, cast bf16
        v = f32(_bf16(xT.T @ wv + vb))
        v = v.reshape(T, H, D)  # token-major bf16
        att_oT = np.empty((256, T), np.float32)
        for b in range(B_LOC):
            for g in range(2):
                for half in range(2):
                    it = b * 4 + half * 2 + g
                    dve_tiles = DVE_TILES_A if it % 2 == 0 else DVE_TILES_B
                    qsl = slice(b * S + half * 512, b * S + half * 512 + 512)
                    o_acc = np.zeros((128, 512), np.float32)
                    den_acc = np.zeros((4, 512), np.float32)
                    for j in range(8):
                        ksl = slice(b * S + j * 128, b * S + j * 128 + 128)
                        for p in range(2):
                            for e in range(2):
                                hl = 2 * p + e
                                psl = slice(g * 128 + 32 * hl, g * 128 + 32 * hl + 32)
                                kc = qkr[256 + psl.start : 256 + psl.stop, ksl]
                                qc = qkr[psl, qsl]
                                scores = kc.T @ qc  # [128, 512]
                                if 2 * j + p in dve_tiles:
                                    ex = _exp16_ref(scores, None, C0P, C1P, C2P)
                                else:
                                    ex = np.exp(scores * SCALE)
                                ex = f32(_bf16(ex))
                                vb_ = v[b * S + j * 128 : b * S + (j + 1) * 128,
                                        4 * g + hl]  # [128, 32]
                                o_acc[32 * hl : 32 * hl + 32] += vb_.T @ ex
                                den_acc[hl] += ex.sum(0)
                    bc = 1.0 / den_acc  # recip_approx ~ exact here
                    o_n = np.empty_like(o_acc)
                    for hl in range(4):
                        o_n[32 * hl : 32 * hl + 32] = (
                            o_acc[32 * hl : 32 * hl + 32] * bc[hl]
                        )
                    att_oT[g * 128 : (g + 1) * 128, qsl] = f32(_bf16(o_n))
        out = f32(_bf16(att_oT)).T @ wo + wob[None, :]
        outs.append(out.reshape(B_LOC, S, E))
    return np.concatenate(outs, 0).astype(np.float32)
